# revision 1
# baseline (speedup 1.0000x reference)
"""Causal GQA self-attention with RoPE for TRN2, 8 NeuronCores.

Problem: B=2, S=2048, D=2048, H=16 q-heads, KV=4 kv-heads, HD=128.

Sharding: core c = (batch b = c//4, kv-group g = c%4). Each core computes
q-heads 4g..4g+3 and kv-head g for batch b:
  qT = (x[b] @ wq[:, 512g:512g+512]).T      via PE, transposed layout
  kT/vT likewise (128-wide slices), RoPE on qT/kT,
  flash-style causal attention in the S^T (keys-on-partitions) layout,
  partial^T = (attn @ wo[512g:512g+512, :]).T
Host sums the 4 partials per batch and transposes back.

Matmuls in fp16 (scores/output accumulate fp32 in PSUM, softmax
denominator accumulated fp32); end-to-end rel err ~5e-4 vs fp32.
"""
import sys

sys.path.insert(0, "/opt/trn_rl_repo")

import numpy as np

import concourse.tile as tile
from concourse import bacc, mybir
from concourse.bass_utils import run_bass_kernel_spmd

F32 = mybir.dt.float32
F16 = mybir.dt.float16
AF = mybir.ActivationFunctionType
OP = mybir.AluOpType

P = 128          # partitions / head dim
S = 2048         # sequence length
D = 2048         # model dim
NH = 4           # q heads per core
QW = NH * P      # q projection width per core (512)
NKD = D // P     # contraction chunks (16)
QCH = 512        # query chunk (free dim of attention matmuls)
NQC = S // QCH   # 4
KCH = P          # key chunk (128, on partitions)
NKC = S // KCH   # 16
SCALE = float(P) ** -0.5


def _host_constants():
    inv = 1.0 / (10000.0 ** (np.arange(0, P, 2, dtype=np.float64) / P))  # [64]
    pos = np.arange(S, dtype=np.float64)
    freqs = pos[:, None] * inv[None, :]                  # [S, 64]
    emb = np.concatenate([freqs, freqs], axis=-1)        # [S, 128]
    cosT = np.cos(emb).T.astype(np.float16).copy()       # [128, S]
    sinT = np.sin(emb).T.astype(np.float16)
    sinT[: P // 2] *= np.float16(-1.0)                   # fold rotate_half sign
    sinT = sinT.copy()
    # causal step masks: mask[p, j, q] = 1 if q >= p + 128*j
    q = np.arange(QCH)[None, None, :]
    p = np.arange(P)[:, None, None]
    j = np.arange(4)[None, :, None]
    masks = (q >= p + KCH * j).astype(np.float16)        # [128, 4, 512]
    ones = np.ones((P, P), dtype=np.float16)
    return cosT, sinT, masks, ones


def build_nc():
    cosT_np, sinT_np, masks_np, ones_np = _host_constants()

    nc = bacc.Bacc(None)
    xT_d = nc.dram_tensor("xT", [D, S], F16, kind="ExternalInput")
    wq_d = nc.dram_tensor("wq", [D, QW], F16, kind="ExternalInput")
    wk_d = nc.dram_tensor("wk", [D, P], F16, kind="ExternalInput")
    wv_d = nc.dram_tensor("wv", [D, P], F16, kind="ExternalInput")
    wo_d = nc.dram_tensor("wo", [QW, D], F16, kind="ExternalInput")
    out_d = nc.dram_tensor("outT", [D, S], F32, kind="ExternalOutput")

    cos_d = nc.inline_tensor(cosT_np, name="cosT")
    sin_d = nc.inline_tensor(sinT_np, name="sinT")
    mask_d = nc.inline_tensor(masks_np, name="masks")
    ones_d = nc.inline_tensor(ones_np, name="onesm")

    # DRAM views with the contraction dim split for SBUF partitions.
    xT_v = xT_d[:].rearrange("(kd p) s -> p kd s", p=P)
    wq_v = wq_d[:].rearrange("(kd p) c -> p kd c", p=P)
    wk_v = wk_d[:].rearrange("(kd p) c -> p kd c", p=P)
    wv_v = wv_d[:].rearrange("(kd p) c -> p kd c", p=P)
    wo_v = wo_d[:].rearrange("(a p) o -> p a o", p=P)

    with tile.TileContext(nc) as tc:
        with tc.tile_pool(name="persist", bufs=1) as pp:
            qT = pp.tile([P, NH, S], F16)        # q^T; attention overwrites
            kT = pp.tile([P, S], F16)
            vT = pp.tile([P, S], F16)
            vK = pp.tile([P, NKC, P], F16)       # V as (kpos, kchunk, hd)
            mask_t = pp.tile([P, 4, QCH], F16)
            ones_t = pp.tile([P, P], F16)        # all-ones: colsum + broadcast
            nc.sync.dma_start(mask_t[:], mask_d[:])
            nc.sync.dma_start(ones_t[:], ones_d[:])

            # attention output overwrites qT in place: slice (h, jq-chunk) is
            # written only after every read of that same slice is done.
            attnT = qT

            # ======== Phase 1: QKV projections (full xT resident) ========
            with tc.tile_pool(name="xp", bufs=1) as xp, \
                 tc.tile_pool(name="p1", bufs=2) as p1, \
                 tc.tile_pool(name="p1c", bufs=1) as p1c, \
                 tc.tile_pool(name="psP", bufs=1, space="PSUM") as psP:
                cos_t = p1c.tile([P, S], F16)
                sin_t = p1c.tile([P, S], F16)

                def rope(dst_ap):
                    sw = p1.tile([P, S], F16, tag="swap")
                    half = P // 2
                    nc.sync.dma_start(sw[:half, :], dst_ap[half:, :])
                    nc.sync.dma_start(sw[half:, :], dst_ap[:half, :])
                    nc.vector.tensor_tensor(sw[:], sw[:], sin_t[:], OP.mult)
                    nc.vector.tensor_tensor(dst_ap, dst_ap, cos_t[:], OP.mult)
                    nc.vector.tensor_tensor(dst_ap, dst_ap, sw[:], OP.add)

                # first-needed weight (k) loads before the bulk x DMAs
                wt_first = p1.tile([P, NKD, P], F16, tag="wt")
                nc.sync.dma_start(wt_first[:], wk_v)
                xf = xp.tile([P, NKD, S], F16)
                for kd in range(NKD):
                    nc.sync.dma_start(xf[:, kd, :], xT_v[:, kd, :])
                nc.sync.dma_start(cos_t[:], cos_d[:])
                nc.sync.dma_start(sin_t[:], sin_d[:])

                # cc order: k, v, then q heads — k/v ready earliest.
                CC = [("k", 0), ("v", 0), ("q", 0), ("q", 1), ("q", 2), ("q", 3)]
                for icc, (kind, hh) in enumerate(CC):
                    if kind == "k":
                        wt = wt_first
                    else:
                        if kind == "q":
                            w_v = wq_v[:, :, hh * P : (hh + 1) * P]
                        else:
                            w_v = wv_v
                        wt = p1.tile([P, NKD, P], F16, tag="wt")
                        nc.sync.dma_start(wt[:], w_v)
                    for jr in range(NQC):
                        ps = psP.tile([P, QCH], F32, tag=f"pj{jr % 2}")
                        for kd in range(NKD):
                            nc.tensor.matmul(
                                ps[:],
                                wt[:, kd, :],
                                xf[:, kd, jr * QCH : (jr + 1) * QCH],
                                start=(kd == 0),
                                stop=(kd == NKD - 1),
                            )
                        if kind == "q":
                            dst = qT[:, hh, jr * QCH : (jr + 1) * QCH]
                        elif kind == "k":
                            dst = kT[:, jr * QCH : (jr + 1) * QCH]
                        else:
                            dst = vT[:, jr * QCH : (jr + 1) * QCH]
                        nc.scalar.copy(out=dst, in_=ps[:])
                    if kind == "k":
                        rope(kT[:])
                    elif kind == "v":
                        nc.sync.dma_start_transpose(vK[:], vT[:])
                    else:
                        rope(qT[:, hh, :])

            # wo prefetched during attention so phase 3 starts immediately
            p3w_cm = tc.tile_pool(name="p3w", bufs=1)
            p3w = p3w_cm.__enter__()
            wo_t = p3w.tile([P, NH, D], F16)
            nc.sync.dma_start(wo_t[:], wo_v)

            # ======== Phase 2: attention ==============================
            # Denominator accumulates in PSUM via a per-block all-ones
            # matmul (broadcast colsum) — no cross-engine chains. Emission
            # is software-pipelined: QK of pair i+1 goes into the PE
            # stream before PV/dsum of pair i, hiding the exp latency.
            with tc.tile_pool(name="p2", bufs=6) as p2, \
                 tc.tile_pool(name="psAtt", bufs=1, space="PSUM") as psAtt:
                it = 0
                for h in range(NH):
                    for jq in range(NQC - 1, -1, -1):
                        nkc = 4 * (jq + 1)
                        qs = qT[:, h, jq * QCH : (jq + 1) * QCH]
                        ops = psAtt.tile([P, QCH], F32, tag=f"o{it % 3}")
                        dps = psAtt.tile([P, QCH], F32, tag="d")
                        it += 1
                        npair = nkc // 2

                        def emit_qk(ip):
                            kc0 = 2 * ip
                            sps = psAtt.tile(
                                [P, 2 * QCH], F32, tag=f"s{ip % 2}",
                                name=f"sps{ip % 2}",
                            )
                            for k2 in range(2):
                                nc.tensor.matmul(
                                    sps[:, k2 * QCH : (k2 + 1) * QCH],
                                    kT[:, (kc0 + k2) * P : (kc0 + k2 + 1) * P],
                                    qs,
                                    start=True,
                                    stop=True,
                                )
                            return sps

                        sps_cur = emit_qk(0)
                        for ip in range(npair):
                            kc0 = 2 * ip
                            pT = p2.tile([P, 2 * QCH], F16, tag="pT")
                            nc.scalar.activation(
                                pT[:], sps_cur[:], AF.Exp, scale=SCALE
                            )
                            jd0 = kc0 - 4 * jq
                            if jd0 >= 0:
                                # both halves on the diagonal band
                                nc.vector.tensor_tensor(
                                    pT[:],
                                    pT[:],
                                    mask_t[:, jd0 : jd0 + 2, :],
                                    OP.mult,
                                )
                            if ip + 1 < npair:
                                sps_cur = emit_qk(ip + 1)
                            for k2 in range(2):
                                kc = kc0 + k2
                                pslice = pT[:, k2 * QCH : (k2 + 1) * QCH]
                                nc.tensor.matmul(
                                    ops[:],
                                    vK[:, kc, :],
                                    pslice,
                                    start=(kc == 0),
                                    stop=(kc == nkc - 1),
                                )
                                nc.tensor.matmul(
                                    dps[:],
                                    ones_t[:],
                                    pslice,
                                    start=(kc == 0),
                                    stop=(kc == nkc - 1),
                                )
                        dib = p2.tile([P, QCH], F32, tag="dib")
                        nc.vector.reciprocal_approx_fast(dib[:], dps[:])
                        nc.vector.tensor_tensor(
                            attnT[:, h, jq * QCH : (jq + 1) * QCH],
                            ops[:],
                            dib[:],
                            OP.mult,
                        )

            # ======== Phase 3: output projection =========================
            with tc.tile_pool(name="p3", bufs=4) as p3, \
                 tc.tile_pool(name="psB", bufs=1, space="PSUM") as psB:
                for oc in range(D // P):
                    pos = [
                        psB.tile([P, QCH], F32, tag=f"po{jq}", name=f"po{jq}")
                        for jq in range(NQC)
                    ]
                    for a in range(NH):
                        for jq in range(NQC):
                            nc.tensor.matmul(
                                pos[jq][:],
                                wo_t[:, a, oc * P : (oc + 1) * P],
                                attnT[:, a, jq * QCH : (jq + 1) * QCH],
                                start=(a == 0),
                                stop=(a == NH - 1),
                            )
                    for jq in range(NQC):
                        ot = p3.tile([P, QCH], F32, tag="ot")
                        if jq % 2 == 0:
                            nc.scalar.copy(out=ot[:], in_=pos[jq][:])
                        else:
                            nc.vector.tensor_copy(out=ot[:], in_=pos[jq][:])
                        nc.sync.dma_start(
                            out_d[oc * P : (oc + 1) * P, jq * QCH : (jq + 1) * QCH],
                            ot[:],
                        )
            p3w_cm.__exit__(None, None, None)

    nc.finalize()
    return nc


_NC = None


def _get_nc():
    global _NC
    if _NC is None:
        _NC = build_nc()
    return _NC


def make_in_maps(x, wq, wk, wv, wo):
    x = np.asarray(x, dtype=np.float32)
    f16 = np.float16
    in_maps = []
    for c in range(8):
        b, g = c // 4, c % 4
        in_maps.append(
            {
                "xT": np.ascontiguousarray(x[b].T).astype(f16),
                "wq": np.asarray(wq[:, QW * g : QW * (g + 1)], dtype=f16),
                "wk": np.asarray(wk[:, P * g : P * (g + 1)], dtype=f16),
                "wv": np.asarray(wv[:, P * g : P * (g + 1)], dtype=f16),
                "wo": np.asarray(wo[QW * g : QW * (g + 1), :], dtype=f16),
            }
        )
    return in_maps


def kernel(x, wq, wk, wv, wo):
    nc = _get_nc()
    in_maps = make_in_maps(x, wq, wk, wv, wo)
    res = run_bass_kernel_spmd(nc, in_maps, list(range(8)))
    parts = [res.results[c]["outT"] for c in range(8)]
    out = np.stack(
        [
            (parts[0] + parts[1] + parts[2] + parts[3]).T,
            (parts[4] + parts[5] + parts[6] + parts[7]).T,
        ]
    ).astype(np.float32)
    return out



# revision 4
# speedup vs baseline: 1.1407x; 1.1407x over previous
"""Causal GQA self-attention with RoPE for TRN2, 8 NeuronCores.

Problem: B=2, S=2048, D=2048, H=16 q-heads, KV=4 kv-heads, HD=128.

Sharding: core c = (batch b = c//4, kv-group g = c%4). Each core computes
q-heads 4g..4g+3 and kv-head g for batch b in the transposed (S^T) layout,
then a partial output projection; host sums the 4 partials per batch.

v3 notes (PE is the bottleneck; keep it saturated end to end):
  - weights are pre-packed on the host into SBUF partition-major layout
    so every DMA descriptor is 4-16KB (a [2048,128] weight loaded
    partition-major otherwise gathers 256B rows).
  - DMA dispatch on the sync queue costs ~650ns per dma_start, so
    dispatches are ordered by need: wk, wv, then x chunks (K/V
    projections run kd-outer, chasing the x stream), then the rest.
  - softmax denominator via DVE accumulation of exp tiles + one
    ones-matmul colsum pair per (head, query block).
  - attention runs jq-outer / h-inner; output-projection matmuls are
    drained into the exp-latency gaps of the attention stream.
  - QK/PV matmuls on the causal diagonal are narrowed to the unmasked
    query range (the skipped region of the exp tile holds stale-but-
    bounded scores, which the mask multiply zeroes anyway).
  - output stored f16; host sums partials in f32.
"""
import sys

sys.path.insert(0, "/opt/trn_rl_repo")

import numpy as np

import concourse.tile as tile
from concourse import bacc, mybir
from concourse.bass_utils import run_bass_kernel_spmd

F32 = mybir.dt.float32
F16 = mybir.dt.float16
AF = mybir.ActivationFunctionType
OP = mybir.AluOpType

P = 128          # partitions / head dim
S = 2048         # sequence length
D = 2048         # model dim
NH = 4           # q heads per core
QW = NH * P      # q projection width per core (512)
NKD = D // P     # contraction chunks (16)
QCH = 512        # query chunk (free dim of attention matmuls)
NQC = S // QCH   # 4
KCH = P          # key chunk (128, on partitions)
NKC = S // KCH   # 16
SCALE = float(P) ** -0.5


def _host_constants():
    inv = 1.0 / (10000.0 ** (np.arange(0, P, 2, dtype=np.float64) / P))  # [64]
    pos = np.arange(S, dtype=np.float64)
    freqs = pos[:, None] * inv[None, :]                  # [S, 64]
    emb = np.concatenate([freqs, freqs], axis=-1)        # [S, 128]
    cosT = np.cos(emb).T.astype(np.float16).copy()       # [128, S]
    sinT = np.sin(emb).T.astype(np.float16)
    sinT[: P // 2] *= np.float16(-1.0)                   # fold rotate_half sign
    sinT = sinT.copy()
    # causal step masks: mask[p, j, q] = 1 if q >= p + 128*j
    q = np.arange(QCH)[None, None, :]
    p = np.arange(P)[:, None, None]
    j = np.arange(4)[None, :, None]
    masks = (q >= p + KCH * j).astype(np.float16)        # [128, 4, 512]
    ones = np.ones((P, P), dtype=np.float16)
    return cosT, sinT, masks, ones


def build_nc():
    cosT_np, sinT_np, masks_np, ones_np = _host_constants()

    nc = bacc.Bacc(None)
    # all weights arrive pre-packed as [128, ...] partition-major arrays
    xT_d = nc.dram_tensor("xT", [D, S], F16, kind="ExternalInput")
    wq_d = nc.dram_tensor("wq", [P, NKD * QW], F16, kind="ExternalInput")
    wk_d = nc.dram_tensor("wk", [P, NKD * P], F16, kind="ExternalInput")
    wv_d = nc.dram_tensor("wv", [P, NKD * P], F16, kind="ExternalInput")
    wo_d = nc.dram_tensor("wo", [P, NH * D], F16, kind="ExternalInput")
    out_d = nc.dram_tensor("outT", [D, S], F16, kind="ExternalOutput")

    cos_d = nc.inline_tensor(cosT_np, name="cosT")
    sin_d = nc.inline_tensor(sinT_np, name="sinT")
    mask_d = nc.inline_tensor(masks_np, name="masks")
    ones_d = nc.inline_tensor(ones_np, name="onesm")

    xT_v = xT_d[:].rearrange("(kd p) s -> p kd s", p=P)

    with tile.TileContext(nc) as tc:
        with tc.tile_pool(name="persist", bufs=1) as pp:
            qT = pp.tile([P, NH, S], F16)        # q^T; attention overwrites
            kT = pp.tile([P, S], F16)
            vT = pp.tile([P, S], F16)
            vK = pp.tile([P, NKC, P], F16)       # V as (kpos, kchunk, hd)
            mask_t = pp.tile([P, 4, QCH], F16)
            ones_t = pp.tile([P, P], F16)        # all-ones for colsum

            # attention output overwrites qT in place: slice (h, jq-chunk) is
            # written only after every read of that same slice is done.
            attnT = qT

            # ======== Phase 1: QKV projections ===========================
            with tc.tile_pool(name="xp", bufs=1) as xp, \
                 tc.tile_pool(name="wp", bufs=1) as wp, \
                 tc.tile_pool(name="p1", bufs=2) as p1, \
                 tc.tile_pool(name="p1c", bufs=1) as p1c, \
                 tc.tile_pool(name="psP", bufs=1, space="PSUM") as psP:
                # dispatch order == need order: wk, wv, x chunks, the rest
                wkt = wp.tile([P, NKD, P], F16)
                wvt = wp.tile([P, NKD, P], F16)
                nc.sync.dma_start(wkt[:], wk_d[:].rearrange("p (kd c) -> p kd c", c=P))
                nc.sync.dma_start(wvt[:], wv_d[:].rearrange("p (kd c) -> p kd c", c=P))
                xf = xp.tile([P, NKD, S], F16)
                for kd in range(NKD):
                    nc.sync.dma_start(xf[:, kd, :], xT_v[:, kd, :])
                cos_t = p1c.tile([P, S], F16)
                sin_t = p1c.tile([P, S], F16)
                nc.sync.dma_start(cos_t[:], cos_d[:])
                nc.sync.dma_start(sin_t[:], sin_d[:])
                wqt = wp.tile([P, NKD, QW], F16)
                nc.sync.dma_start(wqt[:], wq_d[:].rearrange("p (kd c) -> p kd c", c=QW))
                nc.sync.dma_start(mask_t[:], mask_d[:])
                nc.sync.dma_start(ones_t[:], ones_d[:])

                def rope(dst_ap):
                    sw = p1.tile([P, S], F16, tag="swap")
                    half = P // 2
                    nc.sync.dma_start(sw[:half, :], dst_ap[half:, :])
                    nc.sync.dma_start(sw[half:, :], dst_ap[:half, :])
                    nc.vector.tensor_tensor(sw[:], sw[:], sin_t[:], OP.mult)
                    nc.vector.tensor_tensor(dst_ap, dst_ap, cos_t[:], OP.mult)
                    nc.vector.tensor_tensor(dst_ap, dst_ap, sw[:], OP.add)

                # K and V projections, kd-outer, 8 PSUM accumulators
                psK = [psP.tile([P, QCH], F32, tag=f"pk{jr}", name=f"pk{jr}")
                       for jr in range(NQC)]
                psV = [psP.tile([P, QCH], F32, tag=f"pv{jr}", name=f"pv{jr}")
                       for jr in range(NQC)]
                for kd in range(NKD):
                    for jr in range(NQC):
                        nc.tensor.matmul(
                            psK[jr][:], wkt[:, kd, :],
                            xf[:, kd, jr * QCH : (jr + 1) * QCH],
                            start=(kd == 0), stop=(kd == NKD - 1),
                        )
                    for jr in range(NQC):
                        nc.tensor.matmul(
                            psV[jr][:], wvt[:, kd, :],
                            xf[:, kd, jr * QCH : (jr + 1) * QCH],
                            start=(kd == 0), stop=(kd == NKD - 1),
                        )
                for jr in range(NQC):
                    nc.scalar.copy(
                        out=kT[:, jr * QCH : (jr + 1) * QCH], in_=psK[jr][:]
                    )
                # pre-warm the exp table set while ACT is idle-ish
                warm = p1.tile([P, 1], F32, tag="warm")
                nc.scalar.activation(warm[:], psK[0][:, 0:1], AF.Exp, scale=1.0)
                for jr in range(NQC):
                    nc.scalar.copy(
                        out=vT[:, jr * QCH : (jr + 1) * QCH], in_=psV[jr][:]
                    )
                rope(kT[:])
                nc.sync.dma_start_transpose(vK[:], vT[:])

                # Q projections per head, ping-pong PSUM
                for hh in range(NH):
                    for jr in range(NQC):
                        ps = psP.tile([P, QCH], F32, tag=f"pk{jr % 2}",
                                      name=f"pk{jr % 2}")
                        for kd in range(NKD):
                            nc.tensor.matmul(
                                ps[:],
                                wqt[:, kd, hh * P : (hh + 1) * P],
                                xf[:, kd, jr * QCH : (jr + 1) * QCH],
                                start=(kd == 0), stop=(kd == NKD - 1),
                            )
                        nc.scalar.copy(
                            out=qT[:, hh, jr * QCH : (jr + 1) * QCH], in_=ps[:]
                        )
                    rope(qT[:, hh, :])

            # wo prefetched during phase 1 tail / attention start
            p3w_cm = tc.tile_pool(name="p3w", bufs=1)
            p3w = p3w_cm.__enter__()
            wo_t = p3w.tile([P, NH, D], F16)
            nc.sync.dma_start(wo_t[:], wo_d[:].rearrange("p (a o) -> p a o", a=NH))

            # ======== Phase 2: fused attention + output projection =======
            # PSUM budget (8 banks): s0,s1 = 2+2, ops = 1, dps = 1, po = 2.
            with tc.tile_pool(name="p2", bufs=1) as p2, \
                 tc.tile_pool(name="psF", bufs=1, space="PSUM") as psF:

                # pending output-projection emissions, drained into the
                # exp-latency gaps of the attention stream
                jobs = []
                njobs = NQC * (D // P)
                nemitted = [0]

                def drain(n):
                    for _ in range(n):
                        if not jobs:
                            return
                        jobs.pop(0)()

                def make_job(oc, jq):
                    def job():
                        nemitted[0] += 1
                        last = nemitted[0] > njobs - 3
                        po = psF.tile([P, QCH], F32, tag=f"po{oc % 2}",
                                      name=f"po{oc % 2}")
                        for a in range(NH):
                            nc.tensor.matmul(
                                po[:],
                                wo_t[:, a, oc * P : (oc + 1) * P],
                                attnT[:, a, jq * QCH : (jq + 1) * QCH],
                                start=(a == 0), stop=(a == NH - 1),
                            )
                        ot = p2.tile([P, QCH], F16, tag="ot", bufs=4)
                        if oc % 2 == 0:
                            nc.scalar.copy(out=ot[:], in_=po[:])
                        else:
                            nc.vector.tensor_copy(out=ot[:], in_=po[:])
                        dst = out_d[oc * P : (oc + 1) * P,
                                    jq * QCH : (jq + 1) * QCH]
                        if last:
                            # split the tail stores across DMA queues
                            for q4 in range(4):
                                nc.sync.dma_start(
                                    dst[q4 * 32 : (q4 + 1) * 32, :],
                                    ot[q4 * 32 : (q4 + 1) * 32, :],
                                )
                        else:
                            nc.sync.dma_start(dst, ot[:])
                    return job

                for jq in range(NQC):
                    for h in range(NH):
                        nkc = 4 * (jq + 1)
                        npair = nkc // 2
                        qs = qT[:, h, jq * QCH : (jq + 1) * QCH]
                        ops = psF.tile([P, QCH], F32, tag="ops", name="ops")
                        pAcc = p2.tile([P, 2 * QCH], F16,
                                       tag="pAcc", bufs=2, name="pAcc")

                        # query offset of the unmasked range for key chunk
                        # kc (0 off the diagonal); jq0/h0 keeps full width
                        # so the s tiles' first-ever use writes every column
                        def qoff(kc):
                            if jq == 0 and h == 0:
                                return 0
                            return max(0, KCH * (kc - 4 * jq))

                        def emit_qk(ip):
                            kc0 = 2 * ip
                            sps = psF.tile(
                                [P, 2 * QCH], F32, tag=f"s{ip % 2}",
                                name=f"sps{ip % 2}",
                            )
                            for k2 in range(2):
                                off = qoff(kc0 + k2)
                                nc.tensor.matmul(
                                    sps[:, k2 * QCH + off : (k2 + 1) * QCH],
                                    kT[:, (kc0 + k2) * P : (kc0 + k2 + 1) * P],
                                    qs[:, off:],
                                    start=True,
                                    stop=True,
                                )
                            return sps

                        sps_cur = emit_qk(0)
                        for ip in range(npair):
                            kc0 = 2 * ip
                            pT = p2.tile([P, 2 * QCH], F16, tag="pT", bufs=4)
                            nc.scalar.activation(
                                pT[:], sps_cur[:], AF.Exp, scale=SCALE
                            )
                            jd0 = kc0 - 4 * jq
                            if jd0 >= 0:
                                # both halves on the diagonal band
                                nc.vector.tensor_tensor(
                                    pT[:],
                                    pT[:],
                                    mask_t[:, jd0 : jd0 + 2, :],
                                    OP.mult,
                                )
                            if ip + 1 < npair:
                                sps_cur = emit_qk(ip + 1)
                            for k2 in range(2):
                                kc = kc0 + k2
                                off = qoff(kc)
                                nc.tensor.matmul(
                                    ops[:, off:],
                                    vK[:, kc, :],
                                    pT[:, k2 * QCH + off : (k2 + 1) * QCH],
                                    start=(kc == 0),
                                    stop=(kc == nkc - 1),
                                )
                            drain(1)
                            if ip == 0:
                                nc.vector.tensor_copy(out=pAcc[:], in_=pT[:])
                            else:
                                nc.vector.tensor_tensor(
                                    pAcc[:], pAcc[:], pT[:], OP.add
                                )
                        # cross-partition colsum of pAcc -> denominator
                        dps = psF.tile([P, QCH], F32, tag="dps", name="dps")
                        nc.tensor.matmul(
                            dps[:], ones_t[:], pAcc[:, 0:QCH],
                            start=True, stop=False,
                        )
                        nc.tensor.matmul(
                            dps[:], ones_t[:], pAcc[:, QCH : 2 * QCH],
                            start=False, stop=True,
                        )
                        dib = p2.tile([P, QCH], F32, tag="dib", bufs=2)
                        nc.vector.reciprocal_approx_fast(dib[:], dps[:])
                        nc.vector.tensor_tensor(
                            attnT[:, h, jq * QCH : (jq + 1) * QCH],
                            ops[:],
                            dib[:],
                            OP.mult,
                        )
                        drain(2)
                    for oc in range(D // P):
                        jobs.append(make_job(oc, jq))
                drain(len(jobs))
            p3w_cm.__exit__(None, None, None)

    nc.finalize()
    return nc


_NC = None


def _get_nc():
    global _NC
    if _NC is None:
        _NC = build_nc()
    return _NC


def _pack_pm(w):
    """[K, C] f32 -> [128, (K//128)*C] f16 partition-major pack:
    out[p, kd*C + c] = w[kd*128 + p, c]"""
    K, C = w.shape
    kd = K // P
    return np.ascontiguousarray(
        np.asarray(w, dtype=np.float16).reshape(kd, P, C).transpose(1, 0, 2)
    ).reshape(P, kd * C)


def make_in_maps(x, wq, wk, wv, wo):
    x = np.asarray(x, dtype=np.float32)
    in_maps = []
    for c in range(8):
        b, g = c // 4, c % 4
        in_maps.append(
            {
                "xT": np.ascontiguousarray(x[b].T).astype(np.float16),
                "wq": _pack_pm(wq[:, QW * g : QW * (g + 1)]),
                "wk": _pack_pm(wk[:, P * g : P * (g + 1)]),
                "wv": _pack_pm(wv[:, P * g : P * (g + 1)]),
                "wo": _pack_pm(wo[QW * g : QW * (g + 1), :]),
            }
        )
    return in_maps


def kernel(x, wq, wk, wv, wo):
    nc = _get_nc()
    in_maps = make_in_maps(x, wq, wk, wv, wo)
    res = run_bass_kernel_spmd(nc, in_maps, list(range(8)))
    parts = [res.results[c]["outT"].astype(np.float32) for c in range(8)]
    out = np.stack(
        [
            (parts[0] + parts[1] + parts[2] + parts[3]).T,
            (parts[4] + parts[5] + parts[6] + parts[7]).T,
        ]
    ).astype(np.float32)
    return out


# revision 9
# speedup vs baseline: 1.1904x; 1.0436x over previous
"""Causal GQA self-attention with RoPE for TRN2, 8 NeuronCores.

Problem: B=2, S=2048, D=2048, H=16 q-heads, KV=4 kv-heads, HD=128.

Sharding: core c = (batch b = c//4, kv-group g = c%4). Each core computes
q-heads 4g..4g+3 and kv-head g for batch b in the transposed (S^T) layout,
then a partial output projection; host sums the 4 partials per batch.

v4 notes (PE is the bottleneck; keep it saturated end to end):
  - weights pre-packed on host into SBUF partition-major layout (4-16KB
    DMA descriptors); DMA dispatch costs ~650ns each on a queue, so
    loads are dual-dispatched from the sync AND gpsimd queues, ordered
    by need (wk, wv, x chunks, ...). K/V projections run kd-outer,
    chasing the x stream.
  - RoPE rotate-half is a PE permutation matmul per [128,512] chunk
    (engines cannot cross partitions; the old SBUF->SBUF swap DMA had
    ~7us latency and stalled attention start), then 3 narrow DVE ops.
  - softmax denominator via DVE accumulation of exp tiles + one
    ones-matmul colsum pair per (head, query block).
  - causal masking: only the 128-wide diagonal sliver of each diagonal
    key chunk is mask-multiplied (every diagonal chunk uses the same
    triangular [128,128] mask); QK/PV matmuls and the DVE accumulation
    are narrowed to the live query range.
  - attention runs jq-outer / h-inner; output-projection matmuls are
    drained into the exp-latency gaps of the attention stream.
  - output stored f16; host sums partials in f32.
"""
import sys

sys.path.insert(0, "/opt/trn_rl_repo")

import numpy as np

import concourse.tile as tile
from concourse import bacc, mybir
from concourse.bass_utils import run_bass_kernel_spmd

F32 = mybir.dt.float32
F16 = mybir.dt.float16
AF = mybir.ActivationFunctionType
OP = mybir.AluOpType

P = 128          # partitions / head dim
S = 2048         # sequence length
D = 2048         # model dim
NH = 4           # q heads per core
QW = NH * P      # q projection width per core (512)
NKD = D // P     # contraction chunks (16)
QCH = 512        # query chunk (free dim of attention matmuls)
NQC = S // QCH   # 4
KCH = P          # key chunk (128, on partitions)
NKC = S // KCH   # 16
SCALE = float(P) ** -0.5


def _host_constants():
    inv = 1.0 / (10000.0 ** (np.arange(0, P, 2, dtype=np.float64) / P))  # [64]
    pos = np.arange(S, dtype=np.float64)
    freqs = pos[:, None] * inv[None, :]                  # [S, 64]
    emb = np.concatenate([freqs, freqs], axis=-1)        # [S, 128]
    cosT = np.cos(emb).T.astype(np.float16).copy()       # [128, S]
    sinT = np.sin(emb).T.astype(np.float16)
    sinT[: P // 2] *= np.float16(-1.0)                   # fold rotate_half sign
    sinT = sinT.copy()
    # triangular mask for the 128-wide diagonal sliver: m[p, q] = q >= p
    tri = (np.arange(P)[None, :] >= np.arange(P)[:, None]).astype(np.float16)
    # full step masks for the jq0/h0 full-width path: mask[p,j,q] = q >= p+128j
    q = np.arange(QCH)[None, None, :]
    p = np.arange(P)[:, None, None]
    j = np.arange(4)[None, :, None]
    masks = (q >= p + KCH * j).astype(np.float16)        # [128, 4, 512]
    ones = np.ones((P, P), dtype=np.float16)
    # rotate-half permutation: out[m] = in[(m+64) % 128]
    perm = np.zeros((P, P), dtype=np.float16)
    perm[(np.arange(P) + P // 2) % P, np.arange(P)] = 1.0
    return cosT, sinT, tri, masks, ones, perm


def build_nc():
    cosT_np, sinT_np, tri_np, masks_np, ones_np, perm_np = _host_constants()

    nc = bacc.Bacc(None)
    # weights arrive pre-packed as [128, ...] partition-major arrays
    xT_d = nc.dram_tensor("xT", [D, S], F16, kind="ExternalInput")
    wq_d = nc.dram_tensor("wq", [P, NKD * QW], F16, kind="ExternalInput")
    wk_d = nc.dram_tensor("wk", [P, NKD * P], F16, kind="ExternalInput")
    wv_d = nc.dram_tensor("wv", [P, NKD * P], F16, kind="ExternalInput")
    wo_d = nc.dram_tensor("wo", [P, NH * D], F16, kind="ExternalInput")
    out_d = nc.dram_tensor("outT", [D, S], F16, kind="ExternalOutput")

    cos_d = nc.inline_tensor(cosT_np, name="cosT")
    sin_d = nc.inline_tensor(sinT_np, name="sinT")
    tri_d = nc.inline_tensor(tri_np, name="trim")
    mask_d = nc.inline_tensor(masks_np, name="masks")
    ones_d = nc.inline_tensor(ones_np, name="onesm")
    perm_d = nc.inline_tensor(perm_np, name="permm")

    xT_v = xT_d[:].rearrange("(kd p) s -> p kd s", p=P)

    # alternate DMA dispatch between the sync and gpsimd queues
    dq = [0]

    def dma(out, in_):
        eng = nc.sync if dq[0] % 2 == 0 else nc.gpsimd
        dq[0] += 1
        eng.dma_start(out, in_)

    with tile.TileContext(nc) as tc:
        with tc.tile_pool(name="persist", bufs=1) as pp:
            qT = pp.tile([P, NH, S], F16)        # q^T; attention overwrites
            kT = pp.tile([P, S], F16)
            vT = pp.tile([P, S], F16)
            vK = pp.tile([P, NKC, P], F16)       # V as (kpos, kchunk, hd)
            tri_t = pp.tile([P, P], F16)
            mask_t = pp.tile([P, 4, QCH], F16)
            ones_t = pp.tile([P, P], F16)        # all-ones for colsum
            perm_t = pp.tile([P, P], F16)

            # attention output overwrites qT in place: slice (h, jq-chunk) is
            # written only after every read of that same slice is done.
            attnT = qT

            # ======== Phase 1: QKV projections + RoPE ====================
            with tc.tile_pool(name="xp", bufs=1) as xp, \
                 tc.tile_pool(name="wp", bufs=1) as wp, \
                 tc.tile_pool(name="p1", bufs=2) as p1, \
                 tc.tile_pool(name="p1c", bufs=1) as p1c, \
                 tc.tile_pool(name="psP", bufs=1, space="PSUM") as psP:
                # dispatch order == need order: wk, wv, x chunks, the rest
                wkt = wp.tile([P, NKD, P], F16)
                wvt = wp.tile([P, NKD, P], F16)
                nc.sync.dma_start(
                    wkt[:], wk_d[:].rearrange("p (kd c) -> p kd c", c=P))
                nc.gpsimd.dma_start(
                    wvt[:], wv_d[:].rearrange("p (kd c) -> p kd c", c=P))
                xf = xp.tile([P, NKD, S], F16)
                for kd in range(NKD):
                    dma(xf[:, kd, :], xT_v[:, kd, :])
                cos_t = p1c.tile([P, S], F16)
                sin_t = p1c.tile([P, S], F16)
                dma(cos_t[:], cos_d[:])
                dma(sin_t[:], sin_d[:])
                dma(perm_t[:], perm_d[:])
                wqt = wp.tile([P, NKD, QW], F16)
                wq_vv = wq_d[:].rearrange("p (kd c) -> p kd c", c=QW)
                for k4 in range(4):
                    dma(wqt[:, 4 * k4 : 4 * (k4 + 1), :],
                        wq_vv[:, 4 * k4 : 4 * (k4 + 1), :])
                dma(tri_t[:], tri_d[:])
                dma(mask_t[:], mask_d[:])
                dma(ones_t[:], ones_d[:])

                def rope_chunk(dst, rot_ps, cslice):
                    # dst = dst*cos + perm(dst)*sin ; rot_ps holds perm(dst)
                    tmpv = p1.tile([P, QCH], F16, tag="ropet", bufs=3)
                    nc.vector.tensor_tensor(
                        tmpv[:], rot_ps[:], sin_t[:, cslice], OP.mult)
                    nc.vector.tensor_tensor(
                        dst, dst, cos_t[:, cslice], OP.mult)
                    nc.vector.tensor_tensor(dst, dst, tmpv[:], OP.add)

                # K and V projections, kd-outer, 8 PSUM accumulators
                psK = [psP.tile([P, QCH], F32, tag=f"pk{jr}", name=f"pk{jr}")
                       for jr in range(NQC)]
                psV = [psP.tile([P, QCH], F32, tag=f"pv{jr}", name=f"pv{jr}")
                       for jr in range(NQC)]
                for kd in range(NKD):
                    for jr in range(NQC):
                        nc.tensor.matmul(
                            psK[jr][:], wkt[:, kd, :],
                            xf[:, kd, jr * QCH : (jr + 1) * QCH],
                            start=(kd == 0), stop=(kd == NKD - 1),
                        )
                    for jr in range(NQC):
                        nc.tensor.matmul(
                            psV[jr][:], wvt[:, kd, :],
                            xf[:, kd, jr * QCH : (jr + 1) * QCH],
                            start=(kd == 0), stop=(kd == NKD - 1),
                        )
                for jr in range(NQC):
                    nc.scalar.copy(
                        out=kT[:, jr * QCH : (jr + 1) * QCH], in_=psK[jr][:]
                    )
                # pre-warm the exp table set while ACT is idle-ish
                warm = p1.tile([P, 1], F32, tag="warm")
                nc.scalar.activation(warm[:], psK[0][:, 0:1], AF.Exp, scale=1.0)
                for jr in range(NQC):
                    nc.scalar.copy(
                        out=vT[:, jr * QCH : (jr + 1) * QCH], in_=psV[jr][:]
                    )
                # rope kT chunk by chunk (perm matmul reuses freed V banks)
                for jr in range(NQC):
                    cs = slice(jr * QCH, (jr + 1) * QCH)
                    rot = psP.tile([P, QCH], F32, tag=f"pv{jr}",
                                   name=f"pv{jr}")
                    nc.tensor.matmul(rot[:], perm_t[:], kT[:, cs],
                                     start=True, stop=True)
                    rope_chunk(kT[:, cs], rot, cs)
                nc.sync.dma_start_transpose(vK[:], vT[:])

                # Q projections per head, ping-pong PSUM + rope per chunk
                for hh in range(NH):
                    for jr in range(NQC):
                        cs = slice(jr * QCH, (jr + 1) * QCH)
                        ps = psP.tile([P, QCH], F32, tag=f"pk{jr % 2}",
                                      name=f"pk{jr % 2}")
                        for kd in range(NKD):
                            nc.tensor.matmul(
                                ps[:],
                                wqt[:, kd, hh * P : (hh + 1) * P],
                                xf[:, kd, jr * QCH : (jr + 1) * QCH],
                                start=(kd == 0), stop=(kd == NKD - 1),
                            )
                        dst = qT[:, hh, cs]
                        nc.scalar.copy(out=dst, in_=ps[:])
                        rot = psP.tile([P, QCH], F32, tag=f"pk{2 + jr % 2}",
                                       name=f"pk{2 + jr % 2}")
                        nc.tensor.matmul(rot[:], perm_t[:], dst,
                                         start=True, stop=True)
                        rope_chunk(dst, rot, cs)

            # wo prefetched during phase 1 tail / attention start
            p3w_cm = tc.tile_pool(name="p3w", bufs=1)
            p3w = p3w_cm.__enter__()
            wo_t = p3w.tile([P, NH, D], F16)
            wo_vv = wo_d[:].rearrange("p (a o) -> p a o", a=NH)
            dma(wo_t[:, 0:2, :], wo_vv[:, 0:2, :])
            dma(wo_t[:, 2:4, :], wo_vv[:, 2:4, :])

            # ======== Phase 2: fused attention + output projection =======
            # PSUM budget (8 banks): s0,s1 = 2+2, ops = 1, dps = 1, po = 2.
            with tc.tile_pool(name="p2", bufs=1) as p2, \
                 tc.tile_pool(name="psF", bufs=1, space="PSUM") as psF:

                # pending output-projection emissions, drained into the
                # exp-latency gaps of the attention stream
                jobs = []
                njobs = NQC * (D // P)
                nemitted = [0]

                def drain(n):
                    for _ in range(n):
                        if not jobs:
                            return
                        jobs.pop(0)()

                def make_job(oc, jq):
                    def job():
                        nemitted[0] += 1
                        last = nemitted[0] > njobs - 3
                        po = psF.tile([P, QCH], F32, tag=f"po{oc % 2}",
                                      name=f"po{oc % 2}")
                        for a in range(NH):
                            nc.tensor.matmul(
                                po[:],
                                wo_t[:, a, oc * P : (oc + 1) * P],
                                attnT[:, a, jq * QCH : (jq + 1) * QCH],
                                start=(a == 0), stop=(a == NH - 1),
                            )
                        ot = p2.tile([P, QCH], F16, tag="ot", bufs=4)
                        if oc % 2 == 0:
                            nc.scalar.copy(out=ot[:], in_=po[:])
                        else:
                            nc.vector.tensor_copy(out=ot[:], in_=po[:])
                        dst = out_d[oc * P : (oc + 1) * P,
                                    jq * QCH : (jq + 1) * QCH]
                        if last:
                            # split the tail stores across DMA queues
                            for q2 in range(2):
                                dma(dst[q2 * 64 : (q2 + 1) * 64, :],
                                    ot[q2 * 64 : (q2 + 1) * 64, :])
                        else:
                            dma(dst, ot[:])
                    return job

                for jq in range(NQC):
                    for h in range(NH):
                        nkc = 4 * (jq + 1)
                        npair = nkc // 2
                        qs = qT[:, h, jq * QCH : (jq + 1) * QCH]
                        ops = psF.tile([P, QCH], F32, tag="ops", name="ops")
                        pAcc = p2.tile([P, 2 * QCH], F16,
                                       tag="pAcc", bufs=2, name="pAcc")

                        # query offset of the live range for key chunk kc
                        # (0 off the diagonal); jq0/h0 keeps full width so
                        # the s tiles' first-ever use writes every column
                        def qoff(kc):
                            if jq == 0 and h == 0:
                                return 0
                            return max(0, KCH * (kc - 4 * jq))

                        def emit_qk(ip):
                            kc0 = 2 * ip
                            sps = psF.tile(
                                [P, 2 * QCH], F32, tag=f"s{ip % 2}",
                                name=f"sps{ip % 2}",
                            )
                            for k2 in range(2):
                                off = qoff(kc0 + k2)
                                nc.tensor.matmul(
                                    sps[:, k2 * QCH + off : (k2 + 1) * QCH],
                                    kT[:, (kc0 + k2) * P : (kc0 + k2 + 1) * P],
                                    qs[:, off:],
                                    start=True,
                                    stop=True,
                                )
                            return sps

                        sps_cur = emit_qk(0)
                        for ip in range(npair):
                            kc0 = 2 * ip
                            pT = p2.tile([P, 2 * QCH], F16, tag="pT", bufs=6)
                            nc.scalar.activation(
                                pT[:], sps_cur[:], AF.Exp, scale=SCALE
                            )
                            diag = kc0 >= 4 * jq
                            first_pair_init = ip == 0
                            if diag and (first_pair_init or (jq == 0 and h == 0)):
                                # full-width mask (also zeroes dead/stale
                                # columns): needed when the masked tile is
                                # about to initialize pAcc via a full-width
                                # copy, and for jq0/h0 whose full-width
                                # matmuls leave live garbage off-range
                                nc.vector.tensor_tensor(
                                    pT[:], pT[:],
                                    mask_t[:, kc0 : kc0 + 2, :], OP.mult,
                                )
                            elif diag:
                                # mask only the 128-wide diagonal slivers
                                for k2 in range(2):
                                    off = qoff(kc0 + k2)
                                    sl = slice(k2 * QCH + off,
                                               k2 * QCH + off + KCH)
                                    nc.vector.tensor_tensor(
                                        pT[:, sl], pT[:, sl], tri_t[:],
                                        OP.mult,
                                    )
                            if ip + 1 < npair:
                                sps_cur = emit_qk(ip + 1)
                            for k2 in range(2):
                                kc = kc0 + k2
                                off = qoff(kc)
                                nc.tensor.matmul(
                                    ops[:, off:],
                                    vK[:, kc, :],
                                    pT[:, k2 * QCH + off : (k2 + 1) * QCH],
                                    start=(kc == 0),
                                    stop=(kc == nkc - 1),
                                )
                            drain(1)
                            if first_pair_init:
                                nc.vector.tensor_copy(out=pAcc[:], in_=pT[:])
                            elif qoff(kc0) == 0 and qoff(kc0 + 1) == 0:
                                nc.vector.tensor_tensor(
                                    pAcc[:], pAcc[:], pT[:], OP.add
                                )
                            else:
                                for k2 in range(2):
                                    off = qoff(kc0 + k2)
                                    sl = slice(k2 * QCH + off, (k2 + 1) * QCH)
                                    nc.vector.tensor_tensor(
                                        pAcc[:, sl], pAcc[:, sl], pT[:, sl],
                                        OP.add,
                                    )
                        # cross-partition colsum of pAcc -> denominator
                        dps = psF.tile([P, QCH], F32, tag="dps", name="dps")
                        nc.tensor.matmul(
                            dps[:], ones_t[:], pAcc[:, 0:QCH],
                            start=True, stop=False,
                        )
                        nc.tensor.matmul(
                            dps[:], ones_t[:], pAcc[:, QCH : 2 * QCH],
                            start=False, stop=True,
                        )
                        dib = p2.tile([P, QCH], F32, tag="dib", bufs=2)
                        nc.vector.reciprocal_approx_fast(dib[:], dps[:])
                        nc.vector.tensor_tensor(
                            attnT[:, h, jq * QCH : (jq + 1) * QCH],
                            ops[:],
                            dib[:],
                            OP.mult,
                        )
                        drain(2)
                    for oc in range(D // P):
                        jobs.append(make_job(oc, jq))
                drain(len(jobs))
            p3w_cm.__exit__(None, None, None)

    nc.finalize()
    return nc


_NC = None


def _get_nc():
    global _NC
    if _NC is None:
        _NC = build_nc()
    return _NC


def _pack_pm(w):
    """[K, C] f32 -> [128, (K//128)*C] f16 partition-major pack:
    out[p, kd*C + c] = w[kd*128 + p, c]"""
    K, C = w.shape
    kd = K // P
    return np.ascontiguousarray(
        np.asarray(w, dtype=np.float16).reshape(kd, P, C).transpose(1, 0, 2)
    ).reshape(P, kd * C)


def make_in_maps(x, wq, wk, wv, wo):
    x = np.asarray(x, dtype=np.float32)
    in_maps = []
    for c in range(8):
        b, g = c // 4, c % 4
        in_maps.append(
            {
                "xT": np.ascontiguousarray(x[b].T).astype(np.float16),
                "wq": _pack_pm(wq[:, QW * g : QW * (g + 1)]),
                "wk": _pack_pm(wk[:, P * g : P * (g + 1)]),
                "wv": _pack_pm(wv[:, P * g : P * (g + 1)]),
                "wo": _pack_pm(wo[QW * g : QW * (g + 1), :]),
            }
        )
    return in_maps


def kernel(x, wq, wk, wv, wo):
    nc = _get_nc()
    in_maps = make_in_maps(x, wq, wk, wv, wo)
    res = run_bass_kernel_spmd(nc, in_maps, list(range(8)))
    parts = [res.results[c]["outT"].astype(np.float32) for c in range(8)]
    out = np.stack(
        [
            (parts[0] + parts[1] + parts[2] + parts[3]).T,
            (parts[4] + parts[5] + parts[6] + parts[7]).T,
        ]
    ).astype(np.float32)
    return out
